# revision 41
# baseline (speedup 1.0000x reference)
"""Trainium2 Bass kernel for nn_Eq1to2 (segment_reduce / equivariant 1->2 layer).

Math (derived from the reference):
  out[n,i,j,s] = leaky_relu( A[n,i,s] + B[n,j,s] + G[n,s]
                             + (i==j) * (D[n,i,s] + Gd[n,s]) ) * mask
with
  A  = x @ W3                       (col term, i-dependent)
  B  = x @ W2                       (row term, j-dependent)
  D  = x @ W1                       (extra diagonal term)
  G  = sum_a agg_a @ W5_a + bias    (per-sample constant)
  Gd = sum_a agg_a @ W4_a           (per-sample diagonal constant)
where the 20 basis ops collapse to W1..W3 = sums of 4 coef slices each and
per-aggregation W4_a / W5_a; agg_a in {sum/49, sum/nobj, max, min} over N.

Sharding: pure data parallel, 1 batch sample per NeuronCore (B=8, 8 cores).

Device strategy per core (output tile [i=128 part, (j,s)=8192 free] fp32):
  - ONE fp16 K=65 matmul per 512-col chunk: lhsT=[xT; ones],
    rhs=[W3 tiled x128 (host-pretiled, loaded straight from DRAM in 4
    pieces split across both HWDGE rings); flat(B + G + bias) row
    written in place by a partition-gather DMA].
  - G / Gd accumulate straight into the B'/diag PSUM via 3 matmuls with
    a stride-0-broadcast agg column as lhsT.
  - eviction fuses leaky_relu: one ACT Lrelu(alpha=.01) PSUM->SBUF per
    chunk; a dummy 1-elem Lrelu at kernel start pre-loads the ACT table
    off the critical path. bp_hi's fp16 cast runs on DVE.
  - first two chunks are 512 cols (earlier first bulk), then 1024s.
  - bulk output DMAs ride the Sync HWDGE ring as clean 4 KB packets.
  - the i==j diagonal values (exact fp32 leaky(A+B+D+G+Gd+bias)) go to a
    SEPARATE tiny DRAM output "diag" [128, 64] via one rectangular DMA,
    fully decoupled from the bulk stream (no DRAM-overlap deps, no SWDGE
    traffic, no overwrite ordering); the host scatters it into
    out[i, i, :]. GpSimd/SWDGE is never touched -> cheap teardown drain.
"""

import numpy as np

B, N, C, S = 8, 128, 64, 64
AVG_NOBJ = np.float32(49.0)
NEG = 0.01

# fp16 packed input column layout (single input tensor [128, _BNF])
_BLH = 0         # lhsT [65, 128]: rows 0:64 xT, row 64 ones
_W2B = 128       # [65, 64]: rows W2, row 64 bias
_W1CB = 192      # [65, 64]: rows W1+W2+W3, row 64 bias
_G0 = 256        # 3 x [64, 64] G rhs blocks (W5sm, W5max, W5min)
_GD0 = 448       # 3 x [64, 64] Gd rhs blocks (W4+W5 combined per agg)
_W3R = 640       # W3 tiled x128 [64, 8192]; row 64 = BG row (device-built)
_BNF = 640 + 8192
_NC1 = 128       # critical input DMA: lhsT
_NC2 = 640       # small weight blocks end / W3R start

_CACHE = {}


def _build_nc():
    import concourse.bacc as bacc
    import concourse.bass as bass  # noqa: F401
    import concourse.mybir as mybir
    from concourse.tile import TileContext

    F32 = mybir.dt.float32
    FP16 = mybir.dt.float16
    Alu = mybir.AluOpType
    Act = mybir.ActivationFunctionType

    nc = bacc.Bacc("TRN2", debug=False, num_devices=8)
    inpb_d = nc.dram_tensor("inpb", [128, _BNF], FP16, kind="ExternalInput")
    # main output in fp16 (half the HBM write traffic; adds ~2.4e-4 rel
    # rounding on top of the fp16-matmul ~3.4e-4 — far under the 2e-2
    # gate); the exact-diag sidecar stays fp32
    out_d = nc.dram_tensor("out", [128, N * S], FP16, kind="ExternalOutput")
    diag_d = nc.dram_tensor("diag", [128, S], F32, kind="ExternalOutput")

    # chunk column plan: 512s at both ends (earlier first bulk bytes,
    # shorter last evict->DMA chain), 1024s in the middle
    edges = [0, 512, 1024, 2048, 3072, 4096, 5120, 6144, 7168, 7680, 8192]

    with TileContext(nc) as tc:
        with tc.tile_pool(name="main", bufs=1) as pool, \
             tc.tile_pool(name="pz", bufs=3, space="PSUM") as pzpool, \
             tc.tile_pool(name="psm", bufs=1, space="PSUM") as psmpool:

            inpb = pool.tile([128, _BNF], FP16)
            outbuf = pool.tile([128, N * S], FP16)
            aggs = pool.tile([64, 4], FP16)
            bp_hi = pool.tile([128, 64], FP16)
            dleaky = pool.tile([128, 64], F32)
            scratch = pool.tile([1, 1], F32)

            # critical input (xT) first on the Sync HWDGE ring
            nc.sync.dma_start(out=inpb[:, 0:_NC1], in_=inpb_d[:, 0:_NC1])
            # small weight blocks + 4 pieces of the pretiled W3 block
            # (partitions 0:64), split across both HWDGE rings.
            # w3p0 on sync (lands before the gather needs the queue); the
            # rest on scalar so the sync queue stays clear for the gather
            # pieces and the bulks after.
            nc.scalar.dma_start(out=inpb[:, _NC1:_NC2],
                                in_=inpb_d[:, _NC1:_NC2])
            for p in range(4):
                eng = nc.sync if p < 1 else nc.scalar
                lo = _NC2 + p * 2048
                eng.dma_start(out=inpb[0:64, lo:lo + 2048],
                              in_=inpb_d[0:64, lo:lo + 2048])

            # dummy Lrelu: forces the ACT table load right after the
            # scalar ring's DMA issues (the auto-inserted table load would
            # otherwise delay those issues by ~1.3us)
            nc.scalar.activation(scratch[:, :],
                                 nc.const_aps.scalar_like(1.0, scratch[:, :]),
                                 Act.Lrelu, alpha=NEG)

            lhsT = inpb[0:65, _BLH:_BLH + 128]
            xT = inpb[0:64, _BLH:_BLH + 128]

            # aggregations over N (free dim of xT)
            with nc.allow_low_precision("DVE reduces in fp32; fp16 is only "
                                        "the final rounding of the agg vec"):
                nc.vector.tensor_reduce(out=aggs[:, 0:1], in_=xT,
                                        axis=mybir.AxisListType.X, op=Alu.add)
            nc.vector.tensor_reduce(out=aggs[:, 1:2], in_=xT,
                                    axis=mybir.AxisListType.X, op=Alu.max)
            nc.vector.tensor_reduce(out=aggs[:, 2:3], in_=xT,
                                    axis=mybir.AxisListType.X, op=Alu.min)

            # psum_sm: cols 0:64 = diag z, cols 64:128 = B' + G + bias
            psum_sm = psmpool.tile([128, 128], F32)
            psum_diag = psum_sm[:, 0:64]
            psum_bp = psum_sm[:, 64:128]

            # B' = x @ W2 + bias, then += G_a via agg-broadcast lhsT matmuls
            nc.tensor.matmul(psum_bp, lhsT, inpb[0:65, _W2B:_W2B + 64],
                             start=True, stop=False)
            for a in range(3):
                nc.tensor.matmul(psum_bp,
                                 aggs[:, a:a + 1].broadcast_to([64, 128]),
                                 inpb[0:64, _G0 + 64 * a:_G0 + 64 * (a + 1)],
                                 start=False, stop=(a == 2),
                                 skip_group_check=True)
            # diag z = x @ (W1+W2+W3) + bias + sum_a agg_a @ (W4_a + W5_a)
            nc.tensor.matmul(psum_diag, lhsT, inpb[0:65, _W1CB:_W1CB + 64],
                             start=True, stop=False)
            for a in range(3):
                nc.tensor.matmul(psum_diag,
                                 aggs[:, a:a + 1].broadcast_to([64, 128]),
                                 inpb[0:64, _GD0 + 64 * a:_GD0 + 64 * (a + 1)],
                                 start=False, stop=(a == 2),
                                 skip_group_check=True)

            # BG row: fp16 cast on DVE, partition-gather into rhs row 64.
            # Split 3 ways so chunk 0 unblocks after only 16 descriptors
            # (descriptor gen/fetch, not bytes, dominates gather latency).
            nc.vector.tensor_copy(bp_hi[:, :], psum_bp)
            # gather rides the otherwise-empty SWDGE ring: its descriptor
            # generation (~1.0us) beats the HWDGE issue->flight path
            # (~2.2us), and piece 1 gates the whole matmul train
            for g0, g1 in ((0, 16), (16, 64), (64, 128)):
                nc.gpsimd.dma_start(
                    out=inpb[64:65, _NC2 + g0 * 64:_NC2 + g1 * 64],
                    in_=bp_hi[g0:g1, :])

            # diag path (exact fp32): leaky(A+B+D+G+Gd+bias) -> its own
            # DRAM output, fully off the bulk-write path (scalar ring)
            nc.scalar.activation(dleaky[:, :], psum_diag, Act.Lrelu,
                                 alpha=NEG)
            nc.scalar.dma_start(out=diag_d[:, :], in_=dleaky[:, :])

            nchunks = len(edges) - 1
            for c in range(nchunks):
                lo, hi = edges[c], edges[c + 1]
                w = hi - lo
                pz = pzpool.tile([128, w], F32)
                for h in range(w // 512):
                    o = pz[:, h * 512:(h + 1) * 512]
                    r = inpb[0:65, _NC2 + lo + h * 512:
                             _NC2 + lo + (h + 1) * 512]
                    nc.tensor.matmul(o, lhsT, r, start=True, stop=True)
                sl = slice(lo, hi)
                # eviction split: ACT Lrelu on the front 3/4, DVE on the
                # back 1/4 (plain PSUM->fp16 copy, then 2x-mode fp16 leaky
                # in SBUF — DVE cannot read two PSUM operands; Pool supports
                # neither PSUM access nor scalar_tensor_tensor)
                dv = w // 4
                nc.scalar.activation(outbuf[:, lo:hi - dv],
                                     pz[:, 0:w - dv], Act.Lrelu, alpha=NEG)
                nc.vector.tensor_copy(outbuf[:, hi - dv:hi],
                                      pz[:, w - dv:w])
                nc.vector.scalar_tensor_tensor(
                    out=outbuf[:, hi - dv:hi], in0=outbuf[:, hi - dv:hi],
                    scalar=NEG, in1=outbuf[:, hi - dv:hi],
                    op0=Alu.mult, op1=Alu.max)
                # final bulk issues from the (idle) scalar queue, parallel
                # to the sync queue's issue of the second-to-last bulk
                beng = nc.scalar if c == nchunks - 1 else nc.sync
                beng.dma_start(out=out_d[:, sl], in_=outbuf[:, sl])

    nc.compile()
    return nc


def _get_nc():
    if "nc" not in _CACHE:
        _CACHE["nc"] = _build_nc()
    return _CACHE["nc"]


def _host_pack(inputs, nobj, coefs, bias):
    x = np.asarray(inputs, np.float32)        # [B, N, C]
    nobj = np.asarray(nobj, np.float32)       # [B]
    c = np.asarray(coefs, np.float32)         # [C, S, 20]
    bias = np.asarray(bias, np.float32)       # [S]

    W1 = c[:, :, 0] + c[:, :, 5] + c[:, :, 10] + c[:, :, 15]
    W2 = c[:, :, 1] + c[:, :, 6] + c[:, :, 11] + c[:, :, 16]
    W3 = c[:, :, 2] + c[:, :, 7] + c[:, :, 12] + c[:, :, 17]
    W4 = [c[:, :, 3 + 5 * a] for a in range(4)]   # sum, mean, max, min
    W5 = [c[:, :, 4 + 5 * a] for a in range(4)]

    f16 = np.float16
    W3_t = np.tile(W3.astype(f16), (1, 128))

    in_maps = []
    for n in range(B):
        inpb = np.zeros((128, _BNF), f16)
        inpb[0:64, _BLH:_BLH + 128] = x[n].T.astype(f16)
        inpb[64, _BLH:_BLH + 128] = 1.0
        inpb[0:64, _W2B:_W2B + 64] = W2.astype(f16)
        inpb[64, _W2B:_W2B + 64] = bias.astype(f16)
        inpb[0:64, _W1CB:_W1CB + 64] = (W1 + W2 + W3).astype(f16)
        inpb[64, _W1CB:_W1CB + 64] = bias.astype(f16)
        W4sm = W4[0] / AVG_NOBJ + W4[1] / nobj[n]
        W5sm = W5[0] / AVG_NOBJ + W5[1] / nobj[n]
        gs = [W5sm, W5[2], W5[3]]
        gds = [W4sm + W5sm, W4[2] + W5[2], W4[3] + W5[3]]
        for a in range(3):
            inpb[0:64, _G0 + 64 * a:_G0 + 64 * (a + 1)] = gs[a].astype(f16)
            inpb[0:64, _GD0 + 64 * a:_GD0 + 64 * (a + 1)] = gds[a].astype(f16)
        inpb[0:64, _W3R:_BNF] = W3_t
        in_maps.append({"inpb": inpb})
    return in_maps


def _run(inputs, mask, nobj, coefs, bias, trace=False, **trace_kwargs):
    from concourse.bass_utils import run_bass_kernel_spmd

    in_maps = _host_pack(inputs, nobj, coefs, bias)
    nc = _get_nc()
    res = run_bass_kernel_spmd(nc, in_maps, list(range(B)), trace=trace,
                               **trace_kwargs)
    idx = np.arange(N)
    outs = []
    for i in range(B):
        o = res.results[i]["out"].astype(np.float32).reshape(N, N, S)
        o[idx, idx, :] = res.results[i]["diag"]
        outs.append(o)
    out = np.stack(outs)
    m = np.asarray(mask, np.float32)
    if not np.all(m == 1.0):
        out = out * m  # mask is ones in the reference setup; host fallback
    return out, res


def kernel(inputs, mask, nobj, coefs, bias):
    out, _ = _run(inputs, mask, nobj, coefs, bias, trace=False)
    return out


if __name__ == "__main__":
    rng = np.random.default_rng(0)
    inputs = rng.standard_normal((B, N, C)).astype(np.float32)
    mask = np.ones((B, N, N, 1), np.float32)
    nobj = np.full((B,), 100.0, np.float32)
    coefs = (rng.standard_normal((C, S, 20)) * np.sqrt(2.0 / (C * 20))).astype(np.float32)
    bias = np.zeros((S,), np.float32)
    out = kernel(inputs, mask, nobj, coefs, bias)
    print("out", out.shape, out.dtype, float(np.abs(out).max()))


# revision 42
# speedup vs baseline: 1.0038x; 1.0038x over previous
"""Trainium2 Bass kernel for nn_Eq1to2 (segment_reduce / equivariant 1->2 layer).

Math (derived from the reference):
  out[n,i,j,s] = leaky_relu( A[n,i,s] + B[n,j,s] + G[n,s]
                             + (i==j) * (D[n,i,s] + Gd[n,s]) ) * mask
with
  A  = x @ W3                       (col term, i-dependent)
  B  = x @ W2                       (row term, j-dependent)
  D  = x @ W1                       (extra diagonal term)
  G  = sum_a agg_a @ W5_a + bias    (per-sample constant)
  Gd = sum_a agg_a @ W4_a           (per-sample diagonal constant)
where the 20 basis ops collapse to W1..W3 = sums of 4 coef slices each and
per-aggregation W4_a / W5_a; agg_a in {sum/49, sum/nobj, max, min} over N.

Sharding: pure data parallel, 1 batch sample per NeuronCore (B=8, 8 cores).

Device strategy per core (output tile [i=128 part, (j,s)=8192 free] fp32):
  - ONE fp16 K=65 matmul per 512-col chunk: lhsT=[xT; ones],
    rhs=[W3 tiled x128 (host-pretiled, loaded straight from DRAM in 4
    pieces split across both HWDGE rings); flat(B + G + bias) row
    written in place by a partition-gather DMA].
  - G / Gd accumulate straight into the B'/diag PSUM via 3 matmuls with
    a stride-0-broadcast agg column as lhsT.
  - eviction fuses leaky_relu: one ACT Lrelu(alpha=.01) PSUM->SBUF per
    chunk; a dummy 1-elem Lrelu at kernel start pre-loads the ACT table
    off the critical path. bp_hi's fp16 cast runs on DVE.
  - first two chunks are 512 cols (earlier first bulk), then 1024s.
  - bulk output DMAs ride the Sync HWDGE ring as clean 4 KB packets.
  - the i==j diagonal values (exact fp32 leaky(A+B+D+G+Gd+bias)) go to a
    SEPARATE tiny DRAM output "diag" [128, 64] via one rectangular DMA,
    fully decoupled from the bulk stream (no DRAM-overlap deps, no SWDGE
    traffic, no overwrite ordering); the host scatters it into
    out[i, i, :]. GpSimd/SWDGE is never touched -> cheap teardown drain.
"""

import numpy as np

B, N, C, S = 8, 128, 64, 64
AVG_NOBJ = np.float32(49.0)
NEG = 0.01

# fp16 packed input column layout (single input tensor [128, _BNF])
_BLH = 0         # lhsT [65, 128]: rows 0:64 xT, row 64 ones
_W2B = 128       # [65, 64]: rows W2, row 64 bias
_W1CB = 192      # [65, 64]: rows W1+W2+W3, row 64 bias
_G0 = 256        # 3 x [64, 64] G rhs blocks (W5sm, W5max, W5min)
_GD0 = 448       # 3 x [64, 64] Gd rhs blocks (W4+W5 combined per agg)
_W3R = 640       # W3 tiled x128 [64, 8192]; row 64 = BG row (device-built)
_BNF = 640 + 8192
_NC1 = 128       # critical input DMA: lhsT
_NC2 = 640       # small weight blocks end / W3R start

_CACHE = {}


def _build_nc():
    import concourse.bacc as bacc
    import concourse.bass as bass  # noqa: F401
    import concourse.mybir as mybir
    from concourse.tile import TileContext

    F32 = mybir.dt.float32
    FP16 = mybir.dt.float16
    Alu = mybir.AluOpType
    Act = mybir.ActivationFunctionType

    nc = bacc.Bacc("TRN2", debug=False, num_devices=8)
    inpb_d = nc.dram_tensor("inpb", [128, _BNF], FP16, kind="ExternalInput")
    # main output in fp16 (half the HBM write traffic; adds ~2.4e-4 rel
    # rounding on top of the fp16-matmul ~3.4e-4 — far under the 2e-2
    # gate); the exact-diag sidecar stays fp32
    out_d = nc.dram_tensor("out", [128, N * S], FP16, kind="ExternalOutput")
    diag_d = nc.dram_tensor("diag", [128, S], F32, kind="ExternalOutput")

    # chunk column plan: 512s at both ends (earlier first bulk bytes,
    # shorter last evict->DMA chain), 1024s in the middle
    edges = [0, 512, 1024, 2048, 3072, 4096, 5120, 6144, 7168, 7680, 8192]

    with TileContext(nc) as tc:
        with tc.tile_pool(name="main", bufs=1) as pool, \
             tc.tile_pool(name="pz", bufs=3, space="PSUM") as pzpool, \
             tc.tile_pool(name="psm", bufs=1, space="PSUM") as psmpool:

            inpb = pool.tile([128, _BNF], FP16)
            outbuf = pool.tile([128, N * S], FP16)
            aggs = pool.tile([64, 4], FP16)
            bp_hi = pool.tile([128, 64], FP16)
            dleaky = pool.tile([128, 64], F32)
            scratch = pool.tile([1, 1], F32)

            # critical input (xT) first on the Sync HWDGE ring
            nc.sync.dma_start(out=inpb[:, 0:_NC1], in_=inpb_d[:, 0:_NC1])
            # small weight blocks + 4 pieces of the pretiled W3 block
            # (partitions 0:64), split across both HWDGE rings.
            # w3p0 on sync (lands before the gather needs the queue); the
            # rest on scalar so the sync queue stays clear for the gather
            # pieces and the bulks after.
            nc.scalar.dma_start(out=inpb[:, _NC1:_NC2],
                                in_=inpb_d[:, _NC1:_NC2])
            for p in range(4):
                eng = nc.sync if p < 1 else nc.scalar
                lo = _NC2 + p * 2048
                eng.dma_start(out=inpb[0:64, lo:lo + 2048],
                              in_=inpb_d[0:64, lo:lo + 2048])

            # dummy Lrelu: forces the ACT table load right after the
            # scalar ring's DMA issues (the auto-inserted table load would
            # otherwise delay those issues by ~1.3us)
            nc.scalar.activation(scratch[:, :],
                                 nc.const_aps.scalar_like(1.0, scratch[:, :]),
                                 Act.Lrelu, alpha=NEG)

            lhsT = inpb[0:65, _BLH:_BLH + 128]
            xT = inpb[0:64, _BLH:_BLH + 128]

            # aggregations over N (free dim of xT)
            with nc.allow_low_precision("DVE reduces in fp32; fp16 is only "
                                        "the final rounding of the agg vec"):
                nc.vector.tensor_reduce(out=aggs[:, 0:1], in_=xT,
                                        axis=mybir.AxisListType.X, op=Alu.add)
            nc.vector.tensor_reduce(out=aggs[:, 1:2], in_=xT,
                                    axis=mybir.AxisListType.X, op=Alu.max)
            nc.vector.tensor_reduce(out=aggs[:, 2:3], in_=xT,
                                    axis=mybir.AxisListType.X, op=Alu.min)

            # psum_sm: cols 0:64 = diag z, cols 64:128 = B' + G + bias
            psum_sm = psmpool.tile([128, 128], F32)
            psum_diag = psum_sm[:, 0:64]
            psum_bp = psum_sm[:, 64:128]

            # B' = x @ W2 + bias, then += G_a via agg-broadcast lhsT matmuls
            nc.tensor.matmul(psum_bp, lhsT, inpb[0:65, _W2B:_W2B + 64],
                             start=True, stop=False)
            for a in range(3):
                nc.tensor.matmul(psum_bp,
                                 aggs[:, a:a + 1].broadcast_to([64, 128]),
                                 inpb[0:64, _G0 + 64 * a:_G0 + 64 * (a + 1)],
                                 start=False, stop=(a == 2),
                                 skip_group_check=True)
            # diag z = x @ (W1+W2+W3) + bias + sum_a agg_a @ (W4_a + W5_a)
            nc.tensor.matmul(psum_diag, lhsT, inpb[0:65, _W1CB:_W1CB + 64],
                             start=True, stop=False)
            for a in range(3):
                nc.tensor.matmul(psum_diag,
                                 aggs[:, a:a + 1].broadcast_to([64, 128]),
                                 inpb[0:64, _GD0 + 64 * a:_GD0 + 64 * (a + 1)],
                                 start=False, stop=(a == 2),
                                 skip_group_check=True)

            # BG row: fp16 cast on DVE, partition-gather into rhs row 64.
            # Split 3 ways so chunk 0 unblocks after only 16 descriptors
            # (descriptor gen/fetch, not bytes, dominates gather latency).
            nc.vector.tensor_copy(bp_hi[:, :], psum_bp)
            for g0, g1 in ((0, 16), (16, 64), (64, 128)):
                nc.sync.dma_start(
                    out=inpb[64:65, _NC2 + g0 * 64:_NC2 + g1 * 64],
                    in_=bp_hi[g0:g1, :])

            # diag path (exact fp32): leaky(A+B+D+G+Gd+bias) -> its own
            # DRAM output, fully off the bulk-write path (scalar ring)
            nc.scalar.activation(dleaky[:, :], psum_diag, Act.Lrelu,
                                 alpha=NEG)
            nc.scalar.dma_start(out=diag_d[:, :], in_=dleaky[:, :])

            nchunks = len(edges) - 1
            for c in range(nchunks):
                lo, hi = edges[c], edges[c + 1]
                w = hi - lo
                pz = pzpool.tile([128, w], F32)
                for h in range(w // 512):
                    o = pz[:, h * 512:(h + 1) * 512]
                    r = inpb[0:65, _NC2 + lo + h * 512:
                             _NC2 + lo + (h + 1) * 512]
                    nc.tensor.matmul(o, lhsT, r, start=True, stop=True)
                sl = slice(lo, hi)
                # eviction split: ACT Lrelu on the front 3/4, DVE on the
                # back 1/4 (plain PSUM->fp16 copy, then 2x-mode fp16 leaky
                # in SBUF — DVE cannot read two PSUM operands; Pool supports
                # neither PSUM access nor scalar_tensor_tensor)
                dv = w // 4
                nc.scalar.activation(outbuf[:, lo:hi - dv],
                                     pz[:, 0:w - dv], Act.Lrelu, alpha=NEG)
                nc.vector.tensor_copy(outbuf[:, hi - dv:hi],
                                      pz[:, w - dv:w])
                nc.vector.scalar_tensor_tensor(
                    out=outbuf[:, hi - dv:hi], in0=outbuf[:, hi - dv:hi],
                    scalar=NEG, in1=outbuf[:, hi - dv:hi],
                    op0=Alu.mult, op1=Alu.max)
                # final bulk issues from the (idle) scalar queue, parallel
                # to the sync queue's issue of the second-to-last bulk
                beng = nc.scalar if c == nchunks - 1 else nc.sync
                beng.dma_start(out=out_d[:, sl], in_=outbuf[:, sl])

    nc.compile()
    return nc


def _get_nc():
    if "nc" not in _CACHE:
        _CACHE["nc"] = _build_nc()
    return _CACHE["nc"]


def _host_pack(inputs, nobj, coefs, bias):
    x = np.asarray(inputs, np.float32)        # [B, N, C]
    nobj = np.asarray(nobj, np.float32)       # [B]
    c = np.asarray(coefs, np.float32)         # [C, S, 20]
    bias = np.asarray(bias, np.float32)       # [S]

    W1 = c[:, :, 0] + c[:, :, 5] + c[:, :, 10] + c[:, :, 15]
    W2 = c[:, :, 1] + c[:, :, 6] + c[:, :, 11] + c[:, :, 16]
    W3 = c[:, :, 2] + c[:, :, 7] + c[:, :, 12] + c[:, :, 17]
    W4 = [c[:, :, 3 + 5 * a] for a in range(4)]   # sum, mean, max, min
    W5 = [c[:, :, 4 + 5 * a] for a in range(4)]

    f16 = np.float16
    W3_t = np.tile(W3.astype(f16), (1, 128))

    in_maps = []
    for n in range(B):
        inpb = np.zeros((128, _BNF), f16)
        inpb[0:64, _BLH:_BLH + 128] = x[n].T.astype(f16)
        inpb[64, _BLH:_BLH + 128] = 1.0
        inpb[0:64, _W2B:_W2B + 64] = W2.astype(f16)
        inpb[64, _W2B:_W2B + 64] = bias.astype(f16)
        inpb[0:64, _W1CB:_W1CB + 64] = (W1 + W2 + W3).astype(f16)
        inpb[64, _W1CB:_W1CB + 64] = bias.astype(f16)
        W4sm = W4[0] / AVG_NOBJ + W4[1] / nobj[n]
        W5sm = W5[0] / AVG_NOBJ + W5[1] / nobj[n]
        gs = [W5sm, W5[2], W5[3]]
        gds = [W4sm + W5sm, W4[2] + W5[2], W4[3] + W5[3]]
        for a in range(3):
            inpb[0:64, _G0 + 64 * a:_G0 + 64 * (a + 1)] = gs[a].astype(f16)
            inpb[0:64, _GD0 + 64 * a:_GD0 + 64 * (a + 1)] = gds[a].astype(f16)
        inpb[0:64, _W3R:_BNF] = W3_t
        in_maps.append({"inpb": inpb})
    return in_maps


def _run(inputs, mask, nobj, coefs, bias, trace=False, **trace_kwargs):
    from concourse.bass_utils import run_bass_kernel_spmd

    in_maps = _host_pack(inputs, nobj, coefs, bias)
    nc = _get_nc()
    res = run_bass_kernel_spmd(nc, in_maps, list(range(B)), trace=trace,
                               **trace_kwargs)
    idx = np.arange(N)
    outs = []
    for i in range(B):
        o = res.results[i]["out"].astype(np.float32).reshape(N, N, S)
        o[idx, idx, :] = res.results[i]["diag"]
        outs.append(o)
    out = np.stack(outs)
    m = np.asarray(mask, np.float32)
    if not np.all(m == 1.0):
        out = out * m  # mask is ones in the reference setup; host fallback
    return out, res


def kernel(inputs, mask, nobj, coefs, bias):
    out, _ = _run(inputs, mask, nobj, coefs, bias, trace=False)
    return out


if __name__ == "__main__":
    rng = np.random.default_rng(0)
    inputs = rng.standard_normal((B, N, C)).astype(np.float32)
    mask = np.ones((B, N, N, 1), np.float32)
    nobj = np.full((B,), 100.0, np.float32)
    coefs = (rng.standard_normal((C, S, 20)) * np.sqrt(2.0 / (C * 20))).astype(np.float32)
    bias = np.zeros((S,), np.float32)
    out = kernel(inputs, mask, nobj, coefs, bias)
    print("out", out.shape, out.dtype, float(np.abs(out).max()))


# revision 44
# speedup vs baseline: 1.0322x; 1.0283x over previous
"""Trainium2 Bass kernel for nn_Eq1to2 (segment_reduce / equivariant 1->2 layer).

Math (derived from the reference):
  out[n,i,j,s] = leaky_relu( A[n,i,s] + B[n,j,s] + G[n,s]
                             + (i==j) * (D[n,i,s] + Gd[n,s]) ) * mask
with
  A  = x @ W3                       (col term, i-dependent)
  B  = x @ W2                       (row term, j-dependent)
  D  = x @ W1                       (extra diagonal term)
  G  = sum_a agg_a @ W5_a + bias    (per-sample constant)
  Gd = sum_a agg_a @ W4_a           (per-sample diagonal constant)
where the 20 basis ops collapse to W1..W3 = sums of 4 coef slices each and
per-aggregation W4_a / W5_a; agg_a in {sum/49, sum/nobj, max, min} over N.

Sharding: pure data parallel, 1 batch sample per NeuronCore (B=8, 8 cores).

Device strategy per core (output tile [i=128 part, (j,s)=8192 free] fp32):
  - ONE fp16 K=65 matmul per 512-col chunk: lhsT=[xT; ones],
    rhs=[W3 tiled x128 (host-pretiled, loaded straight from DRAM in 4
    pieces split across both HWDGE rings); flat(B + G + bias) row
    written in place by a partition-gather DMA].
  - G / Gd accumulate straight into the B'/diag PSUM via 3 matmuls with
    a stride-0-broadcast agg column as lhsT.
  - eviction fuses leaky_relu: one ACT Lrelu(alpha=.01) PSUM->SBUF per
    chunk; a dummy 1-elem Lrelu at kernel start pre-loads the ACT table
    off the critical path. bp_hi's fp16 cast runs on DVE.
  - first two chunks are 512 cols (earlier first bulk), then 1024s.
  - bulk output DMAs ride the Sync HWDGE ring as clean 4 KB packets.
  - the i==j diagonal values (exact fp32 leaky(A+B+D+G+Gd+bias)) go to a
    SEPARATE tiny DRAM output "diag" [128, 64] via one rectangular DMA,
    fully decoupled from the bulk stream (no DRAM-overlap deps, no SWDGE
    traffic, no overwrite ordering); the host scatters it into
    out[i, i, :]. GpSimd/SWDGE is never touched -> cheap teardown drain.
"""

import numpy as np

B, N, C, S = 8, 128, 64, 64
AVG_NOBJ = np.float32(49.0)
NEG = 0.01

# fp16 packed input column layout (single input tensor [128, _BNF])
_BLH = 0         # lhsT [65, 128]: rows 0:64 xT, row 64 ones
_W2B = 128       # [65, 64]: rows W2, row 64 bias
_W1CB = 192      # [65, 64]: rows W1+W2+W3, row 64 bias
_G0 = 256        # 3 x [64, 64] G rhs blocks (W5sm, W5max, W5min)
_GD0 = 448       # 3 x [64, 64] Gd rhs blocks (W4+W5 combined per agg)
_W3R = 640       # W3 tiled x128 [64, 8192]; row 64 = BG row (device-built)
_BNF = 640 + 8192
_NC1 = 128       # critical input DMA: lhsT
_NC2 = 640       # small weight blocks end / W3R start

_CACHE = {}


def _build_nc():
    import concourse.bacc as bacc
    import concourse.bass as bass  # noqa: F401
    import concourse.mybir as mybir
    from concourse.tile import TileContext

    F32 = mybir.dt.float32
    FP16 = mybir.dt.float16
    Alu = mybir.AluOpType
    Act = mybir.ActivationFunctionType

    nc = bacc.Bacc("TRN2", debug=False, num_devices=8)
    inpb_d = nc.dram_tensor("inpb", [128, _BNF], FP16, kind="ExternalInput")
    # main output in fp16 (half the HBM write traffic; adds ~2.4e-4 rel
    # rounding on top of the fp16-matmul ~3.4e-4 — far under the 2e-2
    # gate); the exact-diag sidecar stays fp32
    out_d = nc.dram_tensor("out", [128, N * S], FP16, kind="ExternalOutput")
    diag_d = nc.dram_tensor("diag", [128, S], F32, kind="ExternalOutput")

    # chunk column plan: 512s at both ends (earlier first bulk bytes,
    # shorter last evict->DMA chain), 1024s in the middle
    edges = [0, 512, 1024, 2048, 3072, 4096, 5120, 6144, 7168, 7680, 8192]

    with TileContext(nc) as tc:
        with tc.tile_pool(name="main", bufs=1) as pool, \
             tc.tile_pool(name="pz", bufs=3, space="PSUM") as pzpool, \
             tc.tile_pool(name="pzl", bufs=1, space="PSUM") as pzlpool, \
             tc.tile_pool(name="psm", bufs=1, space="PSUM") as psmpool:

            inpb = pool.tile([128, _BNF], FP16)
            outbuf = pool.tile([128, N * S], FP16)
            aggs = pool.tile([64, 4], FP16)
            bp_hi = pool.tile([128, 64], FP16)
            dleaky = pool.tile([128, 64], F32)
            scratch = pool.tile([1, 1], F32)

            # critical input (xT) first on the Sync HWDGE ring
            nc.sync.dma_start(out=inpb[:, 0:_NC1], in_=inpb_d[:, 0:_NC1])
            # small weight blocks + 4 pieces of the pretiled W3 block
            # (partitions 0:64), split across both HWDGE rings.
            # w3p0 on sync (lands before the gather needs the queue); the
            # rest on scalar so the sync queue stays clear for the gather
            # pieces and the bulks after.
            nc.scalar.dma_start(out=inpb[:, _NC1:_NC2],
                                in_=inpb_d[:, _NC1:_NC2])
            for p in range(4):
                eng = nc.sync if p < 1 else nc.scalar
                lo = _NC2 + p * 2048
                eng.dma_start(out=inpb[0:64, lo:lo + 2048],
                              in_=inpb_d[0:64, lo:lo + 2048])

            # dummy Lrelu: forces the ACT table load right after the
            # scalar ring's DMA issues (the auto-inserted table load would
            # otherwise delay those issues by ~1.3us)
            nc.scalar.activation(scratch[:, :],
                                 nc.const_aps.scalar_like(1.0, scratch[:, :]),
                                 Act.Lrelu, alpha=NEG)

            lhsT = inpb[0:65, _BLH:_BLH + 128]
            xT = inpb[0:64, _BLH:_BLH + 128]

            # aggregations over N (free dim of xT)
            with nc.allow_low_precision("DVE reduces in fp32; fp16 is only "
                                        "the final rounding of the agg vec"):
                nc.vector.tensor_reduce(out=aggs[:, 0:1], in_=xT,
                                        axis=mybir.AxisListType.X, op=Alu.add)
            nc.vector.tensor_reduce(out=aggs[:, 1:2], in_=xT,
                                    axis=mybir.AxisListType.X, op=Alu.max)
            nc.vector.tensor_reduce(out=aggs[:, 2:3], in_=xT,
                                    axis=mybir.AxisListType.X, op=Alu.min)

            # psum_sm: cols 0:64 = diag z, cols 64:128 = B' + G + bias
            psum_sm = psmpool.tile([128, 128], F32)
            psum_diag = psum_sm[:, 0:64]
            psum_bp = psum_sm[:, 64:128]

            # B' = x @ W2 + bias, then += G_a via agg-broadcast lhsT matmuls
            nc.tensor.matmul(psum_bp, lhsT, inpb[0:65, _W2B:_W2B + 64],
                             start=True, stop=False)
            for a in range(3):
                nc.tensor.matmul(psum_bp,
                                 aggs[:, a:a + 1].broadcast_to([64, 128]),
                                 inpb[0:64, _G0 + 64 * a:_G0 + 64 * (a + 1)],
                                 start=False, stop=(a == 2),
                                 skip_group_check=True)
            # diag z = x @ (W1+W2+W3) + bias + sum_a agg_a @ (W4_a + W5_a)
            nc.tensor.matmul(psum_diag, lhsT, inpb[0:65, _W1CB:_W1CB + 64],
                             start=True, stop=False)
            for a in range(3):
                nc.tensor.matmul(psum_diag,
                                 aggs[:, a:a + 1].broadcast_to([64, 128]),
                                 inpb[0:64, _GD0 + 64 * a:_GD0 + 64 * (a + 1)],
                                 start=False, stop=(a == 2),
                                 skip_group_check=True)

            # BG row: fp16 cast on DVE, partition-gather into rhs row 64.
            # Split 3 ways so chunk 0 unblocks after only 16 descriptors
            # (descriptor gen/fetch, not bytes, dominates gather latency).
            nc.vector.tensor_copy(bp_hi[:, :], psum_bp)
            for g0, g1 in ((0, 16), (16, 64), (64, 128)):
                nc.sync.dma_start(
                    out=inpb[64:65, _NC2 + g0 * 64:_NC2 + g1 * 64],
                    in_=bp_hi[g0:g1, :])

            # diag path (exact fp32): leaky(A+B+D+G+Gd+bias) -> its own
            # DRAM output, fully off the bulk-write path (scalar ring)
            nc.scalar.activation(dleaky[:, :], psum_diag, Act.Lrelu,
                                 alpha=NEG)
            nc.scalar.dma_start(out=diag_d[:, :], in_=dleaky[:, :])

            nchunks = len(edges) - 1
            for c in range(nchunks):
                lo, hi = edges[c], edges[c + 1]
                w = hi - lo
                # the final 512 chunk gets its own 1-bank PSUM slot (the
                # 8th bank) so its matmul never waits for the cycling pool
                # (the slot recycle trails the lagging eviction stream)
                pz = (pzlpool if c == nchunks - 1 else pzpool).tile(
                    [128, w], F32)
                for h in range(w // 512):
                    o = pz[:, h * 512:(h + 1) * 512]
                    r = inpb[0:65, _NC2 + lo + h * 512:
                             _NC2 + lo + (h + 1) * 512]
                    nc.tensor.matmul(o, lhsT, r, start=True, stop=True)
                sl = slice(lo, hi)
                # eviction split: ACT Lrelu on the front 3/4, DVE on the
                # back 1/4 (plain PSUM->fp16 copy, then 2x-mode fp16 leaky
                # in SBUF — DVE cannot read two PSUM operands; Pool supports
                # neither PSUM access nor scalar_tensor_tensor)
                dv = w // 4
                nc.scalar.activation(outbuf[:, lo:hi - dv],
                                     pz[:, 0:w - dv], Act.Lrelu, alpha=NEG)
                nc.vector.tensor_copy(outbuf[:, hi - dv:hi],
                                      pz[:, w - dv:w])
                nc.vector.scalar_tensor_tensor(
                    out=outbuf[:, hi - dv:hi], in0=outbuf[:, hi - dv:hi],
                    scalar=NEG, in1=outbuf[:, hi - dv:hi],
                    op0=Alu.mult, op1=Alu.max)
                # final bulk issues from the (idle) scalar queue, parallel
                # to the sync queue's issue of the second-to-last bulk
                beng = nc.scalar if c == nchunks - 1 else nc.sync
                beng.dma_start(out=out_d[:, sl], in_=outbuf[:, sl])

    nc.compile()
    return nc


def _get_nc():
    if "nc" not in _CACHE:
        _CACHE["nc"] = _build_nc()
    return _CACHE["nc"]


def _host_pack(inputs, nobj, coefs, bias):
    x = np.asarray(inputs, np.float32)        # [B, N, C]
    nobj = np.asarray(nobj, np.float32)       # [B]
    c = np.asarray(coefs, np.float32)         # [C, S, 20]
    bias = np.asarray(bias, np.float32)       # [S]

    W1 = c[:, :, 0] + c[:, :, 5] + c[:, :, 10] + c[:, :, 15]
    W2 = c[:, :, 1] + c[:, :, 6] + c[:, :, 11] + c[:, :, 16]
    W3 = c[:, :, 2] + c[:, :, 7] + c[:, :, 12] + c[:, :, 17]
    W4 = [c[:, :, 3 + 5 * a] for a in range(4)]   # sum, mean, max, min
    W5 = [c[:, :, 4 + 5 * a] for a in range(4)]

    f16 = np.float16
    W3_t = np.tile(W3.astype(f16), (1, 128))

    in_maps = []
    for n in range(B):
        inpb = np.zeros((128, _BNF), f16)
        inpb[0:64, _BLH:_BLH + 128] = x[n].T.astype(f16)
        inpb[64, _BLH:_BLH + 128] = 1.0
        inpb[0:64, _W2B:_W2B + 64] = W2.astype(f16)
        inpb[64, _W2B:_W2B + 64] = bias.astype(f16)
        inpb[0:64, _W1CB:_W1CB + 64] = (W1 + W2 + W3).astype(f16)
        inpb[64, _W1CB:_W1CB + 64] = bias.astype(f16)
        W4sm = W4[0] / AVG_NOBJ + W4[1] / nobj[n]
        W5sm = W5[0] / AVG_NOBJ + W5[1] / nobj[n]
        gs = [W5sm, W5[2], W5[3]]
        gds = [W4sm + W5sm, W4[2] + W5[2], W4[3] + W5[3]]
        for a in range(3):
            inpb[0:64, _G0 + 64 * a:_G0 + 64 * (a + 1)] = gs[a].astype(f16)
            inpb[0:64, _GD0 + 64 * a:_GD0 + 64 * (a + 1)] = gds[a].astype(f16)
        inpb[0:64, _W3R:_BNF] = W3_t
        in_maps.append({"inpb": inpb})
    return in_maps


def _run(inputs, mask, nobj, coefs, bias, trace=False, **trace_kwargs):
    from concourse.bass_utils import run_bass_kernel_spmd

    in_maps = _host_pack(inputs, nobj, coefs, bias)
    nc = _get_nc()
    res = run_bass_kernel_spmd(nc, in_maps, list(range(B)), trace=trace,
                               **trace_kwargs)
    idx = np.arange(N)
    outs = []
    for i in range(B):
        o = res.results[i]["out"].astype(np.float32).reshape(N, N, S)
        o[idx, idx, :] = res.results[i]["diag"]
        outs.append(o)
    out = np.stack(outs)
    m = np.asarray(mask, np.float32)
    if not np.all(m == 1.0):
        out = out * m  # mask is ones in the reference setup; host fallback
    return out, res


def kernel(inputs, mask, nobj, coefs, bias):
    out, _ = _run(inputs, mask, nobj, coefs, bias, trace=False)
    return out


if __name__ == "__main__":
    rng = np.random.default_rng(0)
    inputs = rng.standard_normal((B, N, C)).astype(np.float32)
    mask = np.ones((B, N, N, 1), np.float32)
    nobj = np.full((B,), 100.0, np.float32)
    coefs = (rng.standard_normal((C, S, 20)) * np.sqrt(2.0 / (C * 20))).astype(np.float32)
    bias = np.zeros((S,), np.float32)
    out = kernel(inputs, mask, nobj, coefs, bias)
    print("out", out.shape, out.dtype, float(np.abs(out).max()))
